# revision 3
# baseline (speedup 1.0000x reference)
"""TRN2 Bass kernel for nn_GAT (gnn_message_passing).

3-layer GAT stack: per layer h = relu(x@W+b); e = lrelu(s1[i]+s2[j]) masked by
adj; x += softmax_j(e) @ h.   B=8 graphs, N=2048 nodes, D=128 features.

Sharding: data-parallel over the batch dim — one graph per NeuronCore (8
cores), tiny per-layer weights replicated to every core.

Device algorithm (per core, all layouts transposed: features on partitions,
node index on the free axis):
  lrelu(t) = 0.2*t + 0.8*relu(t), so with t = s1[i] + s2[j]:
    exp(lrelu(t)) = exp(0.2 s1[i]) * exp(0.2 s2[j]) * exp(0.8 relu(t))
  - exp(0.2 s1[i]) cancels between softmax numerator and denominator (no-max
    softmax is numerically safe here: |t| < 5, checked against the module)
  - exp(0.2 s2[j]) folds into the matmul stationaries (h''[j]=relu(h)[j]*e2[j],
    E2MAT[j]=e2[j])
  - per 128x2048 tile the NxN work is one fused DVE tensor_scalar
    (relu(s1bc + s2col)), one ACT Exp, one fp16 mask multiply
  yT[d,i] = sum_j h''[j,d] pp[j,i] and the replicated denominator
  den[i] = sum_j e2[j] pp[j,i] accumulate on the PE in fp32 PSUM;
  1/den via exp(-ln(den)) plus one Newton step; xT += yT * (1/den).

Host side (the sharding step): x -> xT, adj -> maskT fp16, outputs come back
transposed and are flipped on the host. All device DMA is contiguous (the
transposed-access DMA path measures ~5 GB/s on this setup, so layout changes
live on the host).
"""

import numpy as np

B, N, D, L = 8, 2048, 128, 3
NT = N // 128
NCH = N // 512


def _apply_tilefix():
    """This walrus build rejects >1 sync wait on an instruction; the stock
    Tile exit-drain carries several. Spread them across single-wait NOPs."""
    import concourse.tile as tile_mod
    from concourse import mybir

    def _patched_drain_and_barrier(self, tick_clock, wait_clock):
        from concourse.tile import ScopedClock

        drain_inst = self.nc.sync.drain()
        wait_clock.add_sem_waits(
            drain_inst.ins, ScopedClock({None: tick_clock.global_clock})
        )
        si = drain_inst.ins.sync_info
        if si is not None and len(si.on_wait) > 1:
            extra = list(si.on_wait[1:])
            del si.on_wait[1:]
            for w in extra:
                nop = self.nc.sync.nop()
                nop.ins.sync_info = mybir.SyncInfo(on_wait=[w], on_update=[])
        self.nc.all_engine_barrier()
        assert self.sems is not None
        popped = self.nc._tile_sem_poison_stack.pop()
        assert popped is self._sem_poison
        self.nc.clear_and_free_semaphores(list(self.sems.allocated().values()))
        self.nc.all_engine_barrier()

    tile_mod.TileContext._drain_and_barrier = _patched_drain_and_barrier


def _split_sync_waits(nc):
    """Hoist extra sync waits onto same-engine NOPs (walrus one-wait limit)."""
    from concourse import mybir

    n = 0
    for fn in nc.m.functions:
        for bb in fn.blocks:
            new_insts = []
            changed = False
            for inst in bb.instructions:
                si = inst.sync_info
                if si is not None and len(si.on_wait) > 1:
                    extra = list(si.on_wait[1:])
                    del si.on_wait[1:]
                    for w in extra:
                        nop = mybir.InstNoOp(name=f"waitsplit-{n}", ins=[], outs=[])
                        n += 1
                        nop.engine = inst.engine
                        nop.sync_info = mybir.SyncInfo(on_wait=[w], on_update=[])
                        new_insts.append(nop)
                    changed = True
                new_insts.append(inst)
            if changed:
                bb.instructions[:] = new_insts
    return n


def build_gat(reps=1, gps_mask_tiles=6):
    """Build the Bass program. reps>1 wraps the body in a For_i (timing)."""
    import contextlib

    import concourse.bass as bass
    import concourse.tile as tile
    from concourse import mybir

    f32 = mybir.dt.float32
    f16 = mybir.dt.float16
    A = mybir.AluOpType
    F = mybir.ActivationFunctionType

    nc = bass.Bass("TRN2", target_bir_lowering=False, debug=False, num_devices=8)

    xt_in = nc.dram_tensor("xt_in", [128, N], f32, kind="ExternalInput").ap()
    maskt = nc.dram_tensor("maskt", [N, N], f16, kind="ExternalInput").ap()
    wg = nc.dram_tensor("wg", [L, D, D], f32, kind="ExternalInput").ap()
    bg = nc.dram_tensor("bg", [L, D], f32, kind="ExternalInput").ap()
    aa = nc.dram_tensor("aa", [L, 2 * D], f32, kind="ExternalInput").ap()
    xt_out = nc.dram_tensor("xt_out", [128, N], f32, kind="ExternalOutput").ap()

    with tile.TileContext(nc) as tc:
        ctx = contextlib.ExitStack()
        with ctx:
            consts = ctx.enter_context(tc.tile_pool(name="consts", bufs=1))
            mask_pool = ctx.enter_context(tc.tile_pool(name="mask", bufs=1))
            xt_pool = ctx.enter_context(tc.tile_pool(name="xt", bufs=2))
            hT_pool = ctx.enter_context(tc.tile_pool(name="hT", bufs=1))
            hh_pool = ctx.enter_context(tc.tile_pool(name="hh", bufs=1))
            vec_pool = ctx.enter_context(tc.tile_pool(name="vec", bufs=1))
            att_pool = ctx.enter_context(tc.tile_pool(name="att", bufs=3))
            norm_pool = ctx.enter_context(tc.tile_pool(name="norm", bufs=1))

            ones_row = consts.tile([1, 128], f32)
            nc.vector.memset(ones_row, 1.0)
            ones128 = consts.tile([128, 128], f16)
            nc.vector.memset(ones128, 1.0)
            Ws, b_cols, a1cols, a2cols, b_rows = [], [], [], [], []
            for l in range(L):
                W = consts.tile([128, 128], f32, tag=f"W{l}")
                nc.gpsimd.dma_start(out=W[:], in_=wg[l])
                Ws.append(W)
                bc = consts.tile([128, 1], f32, tag=f"bc{l}")
                nc.gpsimd.dma_start(
                    out=bc[:], in_=bg[l].rearrange("(d one) -> d one", one=1)
                )
                b_cols.append(bc)
                a1c = consts.tile([128, 1], f32, tag=f"a1{l}")
                nc.gpsimd.dma_start(
                    out=a1c[:], in_=aa[l, 0:D].rearrange("(d one) -> d one", one=1)
                )
                a1cols.append(a1c)
                a2c = consts.tile([128, 1], f32, tag=f"a2{l}")
                nc.gpsimd.dma_start(
                    out=a2c[:],
                    in_=aa[l, D : 2 * D].rearrange("(d one) -> d one", one=1),
                )
                a2cols.append(a2c)
                br = consts.tile([1, 128], f32, tag=f"br{l}")
                nc.gpsimd.dma_start(
                    out=br[:], in_=bg[l].rearrange("(one d) -> one d", one=1)
                )
                b_rows.append(br)

            def body():
                mask_sb = mask_pool.tile([128, NT * N], f16)
                xT = xt_pool.tile([128, N], f32)
                nc.sync.dma_start(out=xT[:], in_=xt_in[:])
                for jt in range(NT):
                    nc.sync.dma_start(
                        out=mask_sb[:, jt * N : (jt + 1) * N],
                        in_=maskt[jt * 128 : (jt + 1) * 128, :],
                    )

                for l in range(L):
                    W = Ws[l]
                    prep_ctx = contextlib.ExitStack()
                    ps_big = prep_ctx.enter_context(
                        tc.tile_pool(name=f"ps_big{l}", bufs=1, space="PSUM")
                    )
                    ps_small = prep_ctx.enter_context(
                        tc.tile_pool(name=f"ps_small{l}", bufs=2, space="PSUM")
                    )
                    ps_col = prep_ctx.enter_context(
                        tc.tile_pool(name=f"ps_col{l}", bufs=1, space="PSUM")
                    )
                    # hT = relu(W.T @ xT + b)
                    hT_ps = ps_big.tile([128, N], f32, tag="big")
                    for c in range(NCH):
                        sl = slice(c * 512, (c + 1) * 512)
                        nc.tensor.matmul(hT_ps[:, sl], W[:], xT[:, sl])
                    hT = hT_pool.tile([128, N], f32)
                    nc.scalar.activation(
                        hT[:], hT_ps[:], F.Relu, bias=b_cols[l][:], scale=1.0
                    )
                    # s2 as columns, e2 = exp(0.2 s2)
                    scols_ps = ps_col.tile([128, NT], f32, tag="scols")
                    for jt in range(NT):
                        nc.tensor.matmul(
                            scols_ps[:, jt : jt + 1],
                            hT[:, jt * 128 : (jt + 1) * 128],
                            a2cols[l][:],
                        )
                    scols = vec_pool.tile([128, NT], f32, tag="scols_sb")
                    nc.scalar.activation(scols[:], scols_ps[:], F.Copy)
                    e2cols = vec_pool.tile([128, NT], f32, tag="e2_sb")
                    nc.scalar.activation(e2cols[:], scols_ps[:], F.Exp, scale=0.2)
                    # s1 row
                    s1row = vec_pool.tile([1, N], f32, tag="s1row")
                    for c in range(NCH):
                        sl = slice(c * 512, (c + 1) * 512)
                        s1_ps = ps_small.tile([1, 512], f32, tag="small")
                        nc.tensor.matmul(s1_ps[:], a1cols[l][:], hT[:, sl])
                        nc.scalar.activation(s1row[:, sl], s1_ps[:], F.Copy)
                    # s1 broadcast across partitions, fp16
                    s1bc_ps = ps_big.tile([128, N], f32, tag="big")
                    for c in range(NCH):
                        sl = slice(c * 512, (c + 1) * 512)
                        nc.tensor.matmul(s1bc_ps[:, sl], ones_row[:], s1row[:, sl])
                    s1bc = vec_pool.tile([128, N], f16, tag="s1bc")
                    nc.scalar.activation(s1bc[:], s1bc_ps[:], F.Copy)
                    # h'' = relu(h)*e2 (fp16) and E2MAT = ones*e2
                    hh = hh_pool.tile([128, NT * 128], f16, tag="hh")
                    em = hh_pool.tile([128, NT * 128], f16, tag="em")
                    for jt in range(NT):
                        sl = slice(jt * 128, (jt + 1) * 128)
                        hpp_ps = ps_small.tile([128, 128], f32, tag="small")
                        nc.tensor.matmul(
                            hpp_ps[:], xT[:, sl], W[:], start=True, stop=False
                        )
                        nc.tensor.matmul(
                            hpp_ps[:], ones_row[:], b_rows[l][:], start=False, stop=True
                        )
                        nc.vector.tensor_scalar(
                            hh[:, sl], hpp_ps[:], 0.0, e2cols[:, jt : jt + 1],
                            A.max, A.mult,
                        )
                        nc.vector.tensor_scalar(
                            em[:, sl], ones128[:], e2cols[:, jt : jt + 1], None, A.mult
                        )
                    prep_ctx.close()
                    # attention: pp = mask * exp(0.8 relu(s1bc + s2col))
                    attn_ctx = contextlib.ExitStack()
                    ps_y = attn_ctx.enter_context(
                        tc.tile_pool(name=f"ps_y{l}", bufs=1, space="PSUM")
                    )
                    ps_d = attn_ctx.enter_context(
                        tc.tile_pool(name=f"ps_d{l}", bufs=1, space="PSUM")
                    )
                    yT_ps = ps_y.tile([128, N], f32, tag="y")
                    den_ps = ps_d.tile([128, N], f32, tag="d")
                    for jt in range(NT):
                        a_t = att_pool.tile([128, N], f16, tag="a_t")
                        nc.vector.tensor_scalar(
                            a_t[:], s1bc[:], scols[:, jt : jt + 1], 0.0, A.add, A.max
                        )
                        q = att_pool.tile([128, N], f16, tag="q")
                        nc.scalar.activation(q[:], a_t[:], F.Exp, scale=0.8)
                        pp = att_pool.tile([128, N], f16, tag="pp")
                        eng = nc.gpsimd if (jt % NT) < gps_mask_tiles else nc.vector
                        eng.tensor_tensor(
                            pp[:], q[:], mask_sb[:, jt * N : (jt + 1) * N], A.mult
                        )
                        hsl = slice(jt * 128, (jt + 1) * 128)
                        for c in range(NCH):
                            sl = slice(c * 512, (c + 1) * 512)
                            nc.tensor.matmul(
                                yT_ps[:, sl], hh[:, hsl], pp[:, sl],
                                start=(jt == 0), stop=(jt == NT - 1),
                            )
                        for c in range(NCH):
                            sl = slice(c * 512, (c + 1) * 512)
                            nc.tensor.matmul(
                                den_ps[:, sl], em[:, hsl], pp[:, sl],
                                start=(jt == 0), stop=(jt == NT - 1),
                            )
                    # 1/den = exp(-ln den) + one Newton step
                    lnd = norm_pool.tile([128, N], f32, tag="nA")
                    nc.scalar.activation(lnd[:], den_ps[:], F.Ln)
                    r0 = norm_pool.tile([128, N], f32, tag="nB")
                    nc.scalar.activation(r0[:], lnd[:], F.Exp, scale=-1.0)
                    t1 = norm_pool.tile([128, N], f32, tag="nC")
                    nc.vector.tensor_tensor(t1[:], den_ps[:], r0[:], A.mult)
                    u = norm_pool.tile([128, N], f32, tag="nA")
                    nc.vector.tensor_scalar(u[:], t1[:], -1.0, 2.0, A.mult, A.add)
                    r1 = norm_pool.tile([128, N], f32, tag="nC")
                    nc.vector.tensor_tensor(r1[:], r0[:], u[:], A.mult)
                    # xT_new = xT + yT * r1
                    ytmp = norm_pool.tile([128, N], f32, tag="nB")
                    nc.vector.tensor_tensor(ytmp[:], yT_ps[:], r1[:], A.mult)
                    xT_new = xt_pool.tile([128, N], f32)
                    nc.vector.tensor_tensor(xT_new[:], ytmp[:], xT[:], A.add)
                    attn_ctx.close()
                    xT = xT_new

                nc.sync.dma_start(out=xt_out[:], in_=xT[:])

            if reps == 1:
                body()
            else:
                with tc.For_i(0, reps, 1):
                    body()

    return nc


def host_prep(x, adj, Wg, bg, attn_a):
    in_maps = []
    for b in range(B):
        in_maps.append(
            {
                "xt_in": np.ascontiguousarray(x[b].T).astype(np.float32),
                "maskt": np.ascontiguousarray(adj[b].T > 0).astype(np.float16),
                "wg": np.ascontiguousarray(Wg, np.float32),
                "bg": np.ascontiguousarray(bg, np.float32),
                "aa": np.ascontiguousarray(attn_a, np.float32),
            }
        )
    return in_maps


def host_post(results):
    return np.stack([results[b]["xt_out"].T for b in range(B)]).astype(np.float32)


def kernel(x, adj, Wg, bg, attn_a):
    x = np.asarray(x)
    adj = np.asarray(adj)
    Wg = np.asarray(Wg)
    bg = np.asarray(bg)
    attn_a = np.asarray(attn_a)

    _apply_tilefix()
    from concourse.bass_utils import run_bass_kernel_spmd

    nc = build_gat(reps=1)
    _split_sync_waits(nc)
    in_maps = host_prep(x, adj, Wg, bg, attn_a)
    res = run_bass_kernel_spmd(nc, in_maps, core_ids=list(range(B)))
    return host_post(res.results)
